# revision 38
# baseline (speedup 1.0000x reference)
"""MLA (low-rank QKV projection + GQA attention) Bass kernel for 8 trn2 cores.

Problem shapes (hardcoded):
  x [B=2, T=2048, D=2048], Wq1 [512,2048], Wq2 [2048,512],
  Wk1/Wv1 [256,2048], Wk2/Wv2 [512,256], Wo [2048,2048]
  HQ=16 q-heads, HKV=4 kv-heads (GROUP=4), DH=128.

v2 (default, zero attn_mask): token-sharded across 8 cores with on-device
collectives, fp16 internals. v3 schedule: warmup collective absorbs ncfw
init, batched DMA triggers, split v-gather, deferred softmax reduction
(sumexp matmul trails by 1 head, R broadcast by 2, recip via
reciprocal_approx_fast, normalization in-place on GPSIMD).
  Core c owns tokens [c*512, (c+1)*512) of the flattened [B*T]; cores 0-3
  cover batch 0, cores 4-7 batch 1.
  phase 1: low-rank qkv projections for the core's tokens only (all heads):
    klowT/vlowT [256,512], kT [512,512], v [512,512] (bt-major), qlowT,
    qT [2048,512]; k/v go to a DRAM bounce and are AllGathered within the
    4-core batch group -> full-batch kT/v. k transfers in fp8_e4m3 (halves
    the first, latency-critical collective; ~6e-3 rel err, gate 2e-2) and
    is upcast to fp16 on arrival; v stays fp16 (fp8 v breaches the gate).
  phase 2: per q-head h (software-pipelined, two heads of scores in
    flight): scores^T = kT_g.T @ qT_h (psum [keys,1024] pairs),
    E = exp(scale*S) fp16, sumexp tree on DVE+Pool folded to [128,512],
    ones-row matmul -> sumexp row, fp16 reciprocal, K=1 broadcast matmul
    -> R [128,512], attnT_h = PV * R (normalized, fused into the
    psum->sbuf copy).
  phase 3: outT[dm,bt] = sum_heads WoT_tile^T @ attnT (WoT streamed from
    DRAM, prefetched during the attention tail), fp16 out; host
    transposes/concats the 8 exact row-slices.

v1 (fallback for nonzero attn_mask): head-sharded, folded projections,
partial Wo outputs summed on host (the original kernel).
"""

import os
import sys
import types

import numpy as np

import concourse.bass as bass
import concourse.tile as tile
from concourse import mybir
from concourse import bass_utils


def _ensure_ntff_hook():
    """If BASS_TRACE=1 is set but this axon build lacks antenv.axon_hooks,
    run_bass_kernel_spmd(trace=True) crashes on the import. Recreate the tiny
    get/set module and register the ctypes NTFF hook so tracing degrades
    gracefully (or works) instead. No-op when the real module exists."""
    try:
        import antenv.axon_hooks  # noqa: F401
        return
    except ImportError:
        pass
    try:
        mod = types.ModuleType("antenv.axon_hooks")
        mod._hook = None
        mod.set_axon_ntff_profile_hook = lambda h: setattr(mod, "_hook", h)
        mod.get_axon_ntff_profile_hook = lambda: mod._hook
        sys.modules["antenv.axon_hooks"] = mod
        import antenv

        antenv.axon_hooks = mod
        try:
            from trn_agent_boot.trn_boot import _ntff_profile_via_ctypes

            so = "/opt/axon/libaxon_pjrt.so"
            if os.path.exists(so):
                hook = _ntff_profile_via_ctypes(so)
                if hook is not None:
                    mod.set_axon_ntff_profile_hook(hook)
                    # the NEFF-dir upload needs bucket access this
                    # container doesn't have; keep artifacts local
                    _orig_upload = bass_utils.upload_artifacts

                    def _safe_upload(tmpdir):
                        try:
                            return _orig_upload(tmpdir)
                        except Exception:
                            return tmpdir

                    bass_utils.upload_artifacts = _safe_upload
        except Exception:
            pass
    except Exception:
        pass


_ensure_ntff_hook()

D_MODEL, HQ, HKV, RQ, RKV = 2048, 16, 4, 512, 256
DH = D_MODEL // HQ            # 128
GROUP = HQ // HKV             # 4
B, T = 2, 2048
BT = B * T                    # 4096
NCORES = 8
HPC = HQ // NCORES            # v1: 2 q-heads per core
SBT = BT // NCORES            # v2: 512 tokens per core
SCALE = 1.0 / np.sqrt(DH)
EXP_BIAS = -3.0               # exp(scale*s - 3): keeps fp16 sumexp small

NK = D_MODEL // 128           # 16 contraction tiles over D
NBT = BT // 512               # 8 bt chunks of 512
NTT = BT // 128               # 32 bt tiles of 128
NQC = T // 512                # 4 query chunks per batch
NKK = T // 128                # 16 key tiles per batch

f32 = mybir.dt.float32
f16 = mybir.dt.float16
f8 = mybir.dt.float8e4


class _TC(tile.TileContext):
    pass


_nop_ctr = [0]


def _split_multi_waits(nc):
    """This walrus build's CoreV3 lowering accepts only ONE sync-wait per
    instruction; move extra waits onto same-engine single-wait nops inserted
    immediately before the instruction."""
    for f in nc.m.functions:
        for bb in f.blocks:
            insts = list(bb.instructions)
            out = []
            changed = False
            for ins in insts:
                si = ins.sync_info
                if si is not None and si.on_wait and len(si.on_wait) > 1:
                    waits = list(si.on_wait)
                    for w in waits[:-1]:
                        _nop_ctr[0] += 1
                        nop = mybir.InstNoOp(
                            name=f"waitsplit_{_nop_ctr[0]}",
                            ins=[],
                            outs=[],
                            engine=ins.engine,
                        )
                        nop.sync_info = mybir.SyncInfo(on_wait=[w], on_update=[])
                        nc.register_instruction(nop)
                        out.append(nop)
                    ins.sync_info = mybir.SyncInfo(
                        on_wait=[waits[-1]], on_update=list(si.on_update)
                    )
                    changed = True
                out.append(ins)
            if changed:
                bb.instructions = out


# ======================================================================
# v2: token-sharded, collectives, fp16
# ======================================================================

def _build_v2():
    nc = bass.Bass(trn_type="TRN2")
    xT = nc.dram_tensor("xT", (D_MODEL, SBT), f16, kind="ExternalInput")
    wq1T = nc.dram_tensor("wq1T", (D_MODEL, RQ), f16, kind="ExternalInput")
    wq2T = nc.dram_tensor("wq2T", (RQ, HQ * DH), f16, kind="ExternalInput")
    wk1T = nc.dram_tensor("wk1T", (D_MODEL, RKV), f16, kind="ExternalInput")
    wk2T = nc.dram_tensor("wk2T", (RKV, HKV * DH), f16, kind="ExternalInput")
    wv1T = nc.dram_tensor("wv1T", (D_MODEL, RKV), f16, kind="ExternalInput")
    wv2T = nc.dram_tensor("wv2T", (RKV, HKV * DH), f16, kind="ExternalInput")
    woT = nc.dram_tensor("woT", (D_MODEL, D_MODEL), f16, kind="ExternalInput")
    ones1 = nc.dram_tensor("ones1", (1, 128), f16, kind="ExternalInput")
    outT = nc.dram_tensor("outT", (D_MODEL, SBT), f16, kind="ExternalOutput")

    Exp = mybir.ActivationFunctionType.Exp
    Copy = mybir.ActivationFunctionType.Copy
    KVD = HKV * DH            # 512
    RG = [[0, 1, 2, 3], [4, 5, 6, 7]]

    with _TC(nc) as tc:
        with (
            tc.tile_pool(name="persist", bufs=1) as persist,
            tc.tile_pool(name="consts", bufs=1) as consts,
            tc.tile_pool(name="dramw", bufs=1, space="DRAM") as dramw,
            tc.tile_pool(name="dramk", bufs=1, space="DRAM") as dramk,
            tc.tile_pool(name="dramv", bufs=1, space="DRAM") as dramv,
            tc.tile_pool(name="dramkg", bufs=1, space="DRAM") as dramkg,
            tc.tile_pool(name="dramvg", bufs=1, space="DRAM") as dramvg,
        ):
            qT_s = persist.tile([128, HQ * SBT], f16)     # head h at cols h*SBT
            # per-group tiles: scores h0 waits only group 0's gather loads
            kT_g = [persist.tile([128, T], f16, name=f"kT_g{g}") for g in range(HKV)]
            v_full = persist.tile([128, NKK * KVD], f16)  # key tile kk at cols kk*KVD
            attnT_s = persist.tile([128, HQ * SBT], f16)  # head h at h*SBT
            ones1_s = consts.tile([1, 128], f16)
            ones_col = consts.tile([128, 1], f16)
            ones4_s = consts.tile([128, 128], f16)
            nc.sync.dma_start(ones1_s[:], ones1[:])
            nc.sync.dma_start(
                ones_col[:], ones1[:].rearrange("o (p x) -> (o p) x", x=1)
            )
            nc.gpsimd.memset(ones4_s[:], 1.0)

            # exactly TWO collectives: ncfw stages ~20-25us per collective in
            # the NEFF, serially, before the first mesh runs -- extra
            # collectives (warmup, split gathers) delay everything.
            # gather the LOW-RANK klow/vlow (256 rows) instead of k/v: the
            # wire shrinks 2x, the gathers trigger ~6us earlier, and the
            # full-batch up-projection afterwards gives the PE useful work
            # during the mesh instead of idling
            # ONE fused collective: klow fp8 in byte-cols 0:512, vlow fp16
            # reinterpreted as fp8 bytes in cols 512:1536 -- saves a whole
            # serial ncfw staging round vs two separate gathers
            kvin = dramk.tile([RKV, 3 * SBT], f8)
            kvg = dramkg.tile([4, RKV, 3 * SBT], f8)

            # ------------- phase 1: low-rank qkv projections -------------
            with (
                tc.tile_pool(name="xin", bufs=1) as xin,
                tc.tile_pool(name="low", bufs=1) as low,
                tc.tile_pool(name="kvtmp", bufs=1) as kvtmp,
                tc.tile_pool(name="wkv", bufs=1) as wkv,
                tc.tile_pool(name="p1", bufs=6, space="PSUM") as p1,
            ):
                xT_s = xin.tile([128, NK * SBT], f16)
                wk1_s = wkv.tile([128, NK * RKV], f16)
                wk2_s = wkv.tile([128, 2 * KVD], f16)
                wv1_s = wkv.tile([128, NK * RKV], f16)
                wv2_s = wkv.tile([128, 2 * KVD], f16)
                wq1_s = wkv.tile([128, NK * RQ], f16)
                wq2a_s = wkv.tile([128, 4 * 1024], f16)
                wq2b_s = wkv.tile([128, 4 * 1024], f16)
                # batched DMAs (each dma_start trigger costs ~0.6us on the
                # issuing queue -- per-tile loads serialize on triggers, not
                # bytes), ordered by first use
                nc.sync.dma_start(
                    wk1_s[:].rearrange("p (t m) -> p t m", t=NK),
                    wk1T[:].rearrange("(t p) m -> p t m", p=128),
                )
                for c4 in range(4):
                    nc.sync.dma_start(
                        xT_s[:, c4 * 4 * SBT : (c4 + 1) * 4 * SBT].rearrange(
                            "p (t m) -> p t m", t=4
                        ),
                        xT[c4 * 512 : (c4 + 1) * 512, :].rearrange(
                            "(t p) m -> p t m", p=128
                        ),
                    )
                nc.sync.dma_start(
                    wk2_s[:].rearrange("p (t m) -> p t m", t=2),
                    wk2T[:].rearrange("(t p) m -> p t m", p=128),
                )
                nc.sync.dma_start(
                    wv1_s[:].rearrange("p (t m) -> p t m", t=NK),
                    wv1T[:].rearrange("(t p) m -> p t m", p=128),
                )
                nc.sync.dma_start(
                    wv2_s[:].rearrange("p (t m) -> p t m", t=2),
                    wv2T[:].rearrange("(t p) m -> p t m", p=128),
                )
                nc.sync.dma_start(
                    wq1_s[:].rearrange("p (t m) -> p t m", t=NK),
                    wq1T[:].rearrange("(t p) m -> p t m", p=128),
                )
                nc.sync.dma_start(
                    wq2a_s[:].rearrange("p (t m) -> p t m", t=4),
                    wq2T[:, 0:1024].rearrange("(t p) m -> p t m", p=128),
                )
                nc.sync.dma_start(
                    wq2b_s[:].rearrange("p (t m) -> p t m", t=4),
                    wq2T[:, 1024:2048].rearrange("(t p) m -> p t m", p=128),
                )

                # klowT [256, SBT] -> fp8 bounce -> gather (one store trigger)
                klow8 = kvtmp.tile([128, 2 * SBT], f8, tag="klow8")
                vlow16 = kvtmp.tile([128, 2 * SBT], f16, tag="vlow16")
                for rt in range(2):
                    ps = p1.tile([128, SBT], f32, tag="ps1")
                    for kd in range(NK):
                        nc.tensor.matmul(
                            ps[:],
                            wk1_s[:, kd * RKV + rt * 128 : kd * RKV + (rt + 1) * 128],
                            xT_s[:, kd * SBT : (kd + 1) * SBT],
                            start=(kd == 0), stop=(kd == NK - 1),
                        )
                    nc.scalar.activation(
                        klow8[:, rt * SBT : (rt + 1) * SBT], ps[:], Copy
                    )
                nc.scalar.dma_start(
                    kvin[:, 0:SBT].rearrange("(t p) m -> p t m", p=128),
                    klow8[:].rearrange("p (t m) -> p t m", t=2),
                )
                # vlowT [256, SBT] -> fp16 bounce -> gather
                for rt in range(2):
                    ps = p1.tile([128, SBT], f32, tag="ps1")
                    for kd in range(NK):
                        nc.tensor.matmul(
                            ps[:],
                            wv1_s[:, kd * RKV + rt * 128 : kd * RKV + (rt + 1) * 128],
                            xT_s[:, kd * SBT : (kd + 1) * SBT],
                            start=(kd == 0), stop=(kd == NK - 1),
                        )
                    nc.scalar.activation(
                        vlow16[:, rt * SBT : (rt + 1) * SBT], ps[:], Copy
                    )
                nc.scalar.dma_start(
                    kvin[:, SBT : 3 * SBT].rearrange("(t p) m -> p t m", p=128),
                    vlow16[:].bitcast(f8).rearrange("p (t m) -> p t m", t=2),
                )
                nc.gpsimd.collective_compute(
                    "AllGather",
                    mybir.AluOpType.bypass,
                    replica_groups=RG,
                    ins=[kvin[:].opt()],
                    outs=[kvg[:].opt()],
                )

                # ---- q projections (overlap with the collectives) ----
                qlow_s = low.tile([128, 4 * SBT], f16)
                for rt in range(4):
                    ps = p1.tile([128, SBT], f32, tag="ps1")
                    for kd in range(NK):
                        nc.tensor.matmul(
                            ps[:],
                            wq1_s[:, kd * RQ + rt * 128 : kd * RQ + (rt + 1) * 128],
                            xT_s[:, kd * SBT : (kd + 1) * SBT],
                            start=(kd == 0), stop=(kd == NK - 1),
                        )
                    nc.vector.tensor_copy(
                        qlow_s[:, rt * SBT : (rt + 1) * SBT], ps[:]
                    )
                for qt in range(16):
                    ps = p1.tile([128, SBT], f32, tag="ps1")
                    wq2_s = wq2a_s if qt < 8 else wq2b_s
                    qo = (qt % 8) * 128
                    for rt in range(4):
                        nc.tensor.matmul(
                            ps[:],
                            wq2_s[:, rt * 1024 + qo : rt * 1024 + qo + 128],
                            qlow_s[:, rt * SBT : (rt + 1) * SBT],
                            start=(rt == 0), stop=(rt == 3),
                        )
                    nc.vector.tensor_copy(
                        qT_s[:, qt * SBT : (qt + 1) * SBT], ps[:]
                    )

                # ---- load gathered klow/vlow (one trigger each), upcast
                # klow on DVE, then up-project the FULL batch kT/v locally.
                # This fills the collective window with PE work and halves
                # the gather wire vs gathering k/v directly. ----
                klow_full8 = low.tile([128, 8 * SBT], f8)   # cols (rt, s, 512)
                klow_full = low.tile([128, 8 * SBT], f16)
                vlow_full = low.tile([128, 8 * SBT], f16)
                for rt in range(2):
                    nc.sync.dma_start(
                        klow_full8[:, rt * 4 * SBT : (rt + 1) * 4 * SBT].rearrange(
                            "p (s m) -> p s m", s=4
                        ),
                        kvg[:, rt * 128 : (rt + 1) * 128, 0:SBT].rearrange(
                            "s p m -> p s m"
                        ),
                    )
                    nc.vector.tensor_copy(
                        klow_full[:, rt * 4 * SBT : (rt + 1) * 4 * SBT],
                        klow_full8[:, rt * 4 * SBT : (rt + 1) * 4 * SBT],
                    )
                    nc.sync.dma_start(
                        vlow_full[:, rt * 4 * SBT : (rt + 1) * 4 * SBT].rearrange(
                            "p (s m) -> p s m", s=4
                        ),
                        kvg[:, rt * 128 : (rt + 1) * 128, SBT : 3 * SBT]
                        .bitcast(f16)
                        .rearrange("s p m -> p s m"),
                    )
                # kT up-projection, group-major so scores h0 unblocks first;
                # psum -> kT_g copies on ACT (idle until the first exp)
                for kt in range(4):
                    for s in range(4):
                        ps = p1.tile([128, SBT], f32, tag="ps1")
                        for rt in range(2):
                            nc.tensor.matmul(
                                ps[:],
                                wk2_s[:, rt * KVD + kt * 128 : rt * KVD + (kt + 1) * 128],
                                klow_full[:, (rt * 4 + s) * SBT : (rt * 4 + s + 1) * SBT],
                                start=(rt == 0), stop=(rt == 1),
                            )
                        nc.scalar.activation(
                            kT_g[kt][:, s * SBT : (s + 1) * SBT], ps[:], Copy
                        )
                # v up-projection; psum -> v_full copies on DVE (emitted
                # ahead of all phase-2 DVE work)
                for s in range(4):
                    for tb in range(4):
                        ps = p1.tile([128, KVD], f32, tag="ps1")
                        for rt in range(2):
                            nc.tensor.matmul(
                                ps[:],
                                vlow_full[:, (rt * 4 + s) * SBT + tb * 128
                                           : (rt * 4 + s) * SBT + (tb + 1) * 128],
                                wv2_s[:, rt * KVD : (rt + 1) * KVD],
                                start=(rt == 0), stop=(rt == 1),
                            )
                        kk = s * 4 + tb
                        nc.vector.tensor_copy(
                            v_full[:, kk * KVD : (kk + 1) * KVD], ps[:]
                        )

            # ---------------- phase 2: attention ----------------
            with (
                tc.tile_pool(name="epool", bufs=26) as epool,
                tc.tile_pool(name="supool", bufs=4) as supool,
                tc.tile_pool(name="rpool", bufs=3) as rpool,
                tc.tile_pool(name="wos", bufs=3) as wos,
            ):
                def load_wo(dmt):
                    wo_s = wos.tile([128, 16 * 128], f16, tag="wo", name=f"wo_{dmt}")
                    nc.sync.dma_start(
                        wo_s[:].rearrange("p (t m) -> p t m", t=16),
                        woT[:, dmt * 128 : (dmt + 1) * 128].rearrange(
                            "(t p) m -> p t m", p=128
                        ),
                    )
                    return wo_s

                p2psum = tc.tile_pool(name="stp", bufs=2, space="PSUM")
                stp = p2psum.__enter__()
                pvp_cm = tc.tile_pool(name="pvp", bufs=2, space="PSUM")
                pvp = pvp_cm.__enter__()
                sump_cm = tc.tile_pool(name="sump", bufs=1, space="PSUM")
                sump = sump_cm.__enter__()
                rp_cm = tc.tile_pool(name="rp", bufs=1, space="PSUM")
                rp = rp_cm.__enter__()

                def emit_scores(h):
                    g = h // GROUP
                    es = []
                    # esum: serial DVE accumulation chain (GPSIMD adds are
                    # ~6x slower than DVE and were on the critical path)
                    esA = supool.tile([128, 1024], f16, tag="esA", name=f"esA_{h}")
                    for j in range(8):
                        ps = stp.tile([128, 1024], f32, tag="st", name=f"st_{h}_{j}")
                        for half in range(2):
                            kk = 2 * j + half
                            nc.tensor.matmul(
                                ps[:, half * 512 : (half + 1) * 512],
                                kT_g[g][:, kk * 128 : (kk + 1) * 128],
                                qT_s[:, h * SBT : (h + 1) * SBT],
                                start=True, stop=True,
                            )
                        e = epool.tile([128, 1024], f16, tag="e", name=f"e_{h}_{j}")
                        nc.scalar.activation(e[:], ps[:], Exp, scale=SCALE)
                        es.append(e)
                        with nc.allow_low_precision(reason="fp16 sumexp"):
                            if j == 1:
                                nc.vector.tensor_add(esA[:], es[0][:], es[1][:])
                            elif j >= 2:
                                nc.vector.tensor_add(esA[:], esA[:], e[:])
                    esum = supool.tile([128, SBT], f16, tag="esum", name=f"esum_{h}")
                    with nc.allow_low_precision(reason="fp16 sumexp"):
                        # fold the two 512-column halves so the ones-matmul
                        # only processes 512 columns
                        nc.vector.tensor_add(
                            esum[:], esA[:, 0:512], esA[:, 512:1024]
                        )
                    return es, esum

                def emit_pv(h, es):
                    g = h // GROUP
                    ps_pv = pvp.tile([128, SBT], f32, tag="pv", name=f"pv_{h}")
                    for kk in range(NKK):
                        nc.tensor.matmul(
                            ps_pv[:],
                            v_full[:, kk * KVD + g * 128 : kk * KVD + (g + 1) * 128],
                            es[kk // 2][:, (kk % 2) * 512 : (kk % 2) * 512 + 512],
                            start=(kk == 0), stop=(kk == NKK - 1),
                        )
                    # unnormalized copy to SBUF right away: frees the PSUM
                    # bank and decouples normalization from the PE pipeline
                    with nc.allow_low_precision(reason="fp16 attn raw"):
                        nc.vector.tensor_copy(
                            attnT_s[:, h * SBT : (h + 1) * SBT], ps_pv[:]
                        )

                # per-head [1,512] sumexp rows are packed 4-to-a-bank at
                # partitions 0/32/64/96 (matmul col tile_position), so the
                # DVE reciprocal -- whose cost is free-size-bound, the same
                # ~3.4us for 1 or 33 partitions -- runs once per HEAD PAIR
                # on a [33,512] slice instead of once per head
                quad_ps = {}
                r16q = {}

                def emit_sum(h, esum):
                    q, qi = h // 4, h % 4
                    if qi == 0:
                        quad_ps[q] = sump.tile([97, SBT], f32, tag="sq", name=f"sq_{q}")
                        r16q[q] = rpool.tile([97, SBT], f16, tag="r16", name=f"r16_{q}")
                    nc.tensor.matmul(
                        quad_ps[q][32 * qi : 32 * qi + 1, :], ones_col[:], esum[:],
                        start=True, stop=True, tile_position=(0, 32 * qi),
                    )
                    if qi == 3:
                        # one reciprocal per QUAD: DVE reciprocal cost is
                        # free-size-bound (~3.4us for 1..97 partitions), so
                        # batching 4 packed rows quarters the per-head cost
                        # and halves how often it can stall the R matmul
                        with nc.allow_low_precision(reason="fp16 softmax recip"):
                            nc.vector.reciprocal(
                                r16q[q][0:97, :], quad_ps[q][0:97, :]
                            )

                def emit_norm(h):
                    # R broadcast via K=1 matmul reading the packed r16 row
                    # (stationary all-ones row at the same base partition);
                    # r16 is three iterations old, so the PE never waits on
                    # the reciprocal; cast on DVE, in-place normalize of the
                    # raw attnT on GPSIMD -- all off the PE critical path
                    q, qi = h // 4, h % 4
                    ps_R = rp.tile([128, SBT], f32, tag="R", name=f"R_{h}")
                    nc.tensor.matmul(
                        ps_R[:],
                        ones4_s[32 * qi : 32 * qi + 1, :],
                        r16q[q][32 * qi : 32 * qi + 1, :],
                        start=True, stop=True, tile_position=(32 * qi, 0),
                    )
                    R_s = rpool.tile([128, SBT], f16, tag="Rs", name=f"Rs_{h}")
                    with nc.allow_low_precision(reason="fp16 softmax normalize"):
                        nc.vector.tensor_copy(R_s[:], ps_R[:])
                        nc.gpsimd.tensor_mul(
                            attnT_s[:, h * SBT : (h + 1) * SBT],
                            attnT_s[:, h * SBT : (h + 1) * SBT],
                            R_s[:],
                        )

                # software pipeline, two heads of scores in flight. PE
                # program order per iteration:
                #   [PV h] [ones h-1] [R h-3] [scores h+2]
                # the sumexp matmul trails its head by one iteration and the
                # R broadcast by three (past the pair-batched reciprocal).
                # emit_sum/emit_norm run BEFORE emit_scores so the 3.4us
                # reciprocal and the R cast sit early in the in-order DVE
                # queue (ahead of the esum chain adds, which trail the exps)
                # -- otherwise the R matmul catches up to the reciprocal and
                # the PE stalls ~1.2us every head pair.
                pipe = [emit_scores(0), emit_scores(1)]
                esums = {}
                wo_tiles = []
                for h in range(HQ):
                    es_cur, esums[h] = pipe.pop(0)
                    emit_pv(h, es_cur)
                    if h >= 1:
                        emit_sum(h - 1, esums.pop(h - 1))
                    if h >= 5:
                        # R trails its head by FIVE iterations: a full
                        # iteration of slack past the quad-batched
                        # reciprocal (which completes at iteration 4q+4 for
                        # heads 4q..4q+3), wherever the Tile scheduler puts
                        # it in the in-order DVE queue
                        emit_norm(h - 5)
                    if h + 2 < HQ:
                        pipe.append(emit_scores(h + 2))
                    if h >= HQ - 3:
                        # prefetch the first WoT column blocks during the
                        # attention tail
                        wo_tiles.append(load_wo(len(wo_tiles)))
                emit_sum(HQ - 1, esums.pop(HQ - 1))
                for h in range(HQ - 5, HQ):
                    emit_norm(h)
                rp_cm.__exit__(None, None, None)
                sump_cm.__exit__(None, None, None)
                pvp_cm.__exit__(None, None, None)
                p2psum.__exit__(None, None, None)

                # ---------------- phase 3: Wo ----------------
                with (
                    tc.tile_pool(name="og", bufs=2) as og,
                    tc.tile_pool(name="wop", bufs=2, space="PSUM") as wop,
                ):
                    for q4 in range(4):
                        o4 = og.tile([128, 4 * SBT], f16, tag="o4", name=f"o4_{q4}")
                        for j4 in range(4):
                            dmt = q4 * 4 + j4
                            wo_s = wo_tiles[dmt]
                            po = wop.tile([128, SBT], f32, tag="po", name=f"po_{dmt}")
                            for adt in range(16):
                                nc.tensor.matmul(
                                    po[:],
                                    wo_s[:, adt * 128 : (adt + 1) * 128],
                                    attnT_s[:, adt * SBT : (adt + 1) * SBT],
                                    start=(adt == 0), stop=(adt == 15),
                                )
                            if dmt + 3 < 16:
                                wo_tiles.append(load_wo(dmt + 3))
                            nc.scalar.activation(
                                o4[:, j4 * SBT : (j4 + 1) * SBT], po[:], Copy
                            )
                        # batched output store: one trigger per 4 dm tiles
                        nc.sync.dma_start(
                            outT[q4 * 512 : (q4 + 1) * 512, :].rearrange(
                                "(t p) m -> p t m", p=128
                            ),
                            o4[:].rearrange("p (t m) -> p t m", t=4),
                        )

    _split_multi_waits(nc)
    return nc


def _prep_inputs_v2(x, Wq1, Wq2, Wk1, Wk2, Wv1, Wv2, Wo):
    xT = np.ascontiguousarray(x.reshape(BT, D_MODEL).T).astype(np.float16)
    wq1T = np.ascontiguousarray(Wq1.T).astype(np.float16)
    wq2T = np.ascontiguousarray(Wq2.T).astype(np.float16)
    wk1T = np.ascontiguousarray(Wk1.T).astype(np.float16)
    wk2T = np.ascontiguousarray(Wk2.T).astype(np.float16)
    wv1T = np.ascontiguousarray(Wv1.T).astype(np.float16)
    wv2T = np.ascontiguousarray(Wv2.T).astype(np.float16)
    woT = np.ascontiguousarray(Wo.T).astype(np.float16)
    ones1 = np.ones((1, 128), np.float16)
    in_maps = []
    for c in range(NCORES):
        in_maps.append({
            "xT": np.ascontiguousarray(xT[:, c * SBT : (c + 1) * SBT]),
            "wq1T": wq1T, "wq2T": wq2T,
            "wk1T": wk1T, "wk2T": wk2T,
            "wv1T": wv1T, "wv2T": wv2T,
            "woT": woT, "ones1": ones1,
        })
    return in_maps


def _run_v2(x, **spmd_kwargs):
    nc = _get_nc_v2()
    in_maps = _prep_inputs_v2(
        x["x"], x["Wq1"], x["Wq2"], x["Wk1"], x["Wk2"], x["Wv1"], x["Wv2"], x["Wo"]
    )
    res = bass_utils.run_bass_kernel_spmd(
        nc, in_maps, core_ids=list(range(NCORES)), **spmd_kwargs
    )
    out = np.empty((BT, D_MODEL), np.float32)
    for c in range(NCORES):
        out[c * SBT : (c + 1) * SBT, :] = res.results[c]["outT"].T.astype(np.float32)
    return out.reshape(B, T, D_MODEL), res


# ======================================================================
# v1: head-sharded fallback (nonzero attn_mask)
# ======================================================================

def _build_v1(mmdt, use_mask):
    nc = bass.Bass(trn_type="TRN2")
    xT = nc.dram_tensor("xT", (D_MODEL, BT), mmdt, kind="ExternalInput")
    wq = nc.dram_tensor("wq", (D_MODEL, HPC * DH), mmdt, kind="ExternalInput")
    wk = nc.dram_tensor("wk", (D_MODEL, DH), mmdt, kind="ExternalInput")
    wv = nc.dram_tensor("wv", (D_MODEL, DH), mmdt, kind="ExternalInput")
    woT = nc.dram_tensor("woT", (HPC * DH, D_MODEL), mmdt, kind="ExternalInput")
    ones = nc.dram_tensor("ones", (128, 1), mmdt, kind="ExternalInput")
    identm = nc.dram_tensor("identm", (128, 128), mmdt, kind="ExternalInput")
    identf = nc.dram_tensor("identf", (128, 128), f32, kind="ExternalInput")
    if use_mask:
        maskT = nc.dram_tensor("maskT", (T, T), f32, kind="ExternalInput")
    out = nc.dram_tensor("out", (BT, D_MODEL), f32, kind="ExternalOutput")

    Exp = mybir.ActivationFunctionType.Exp
    Copy = mybir.ActivationFunctionType.Copy
    with _TC(nc) as tc:
        with (
            tc.tile_pool(name="persist", bufs=1) as persist,
            tc.tile_pool(name="consts", bufs=1) as consts,
        ):
            qT_s = persist.tile([128, HPC * BT], mmdt)
            kT_s = persist.tile([128, BT], mmdt)
            v_s = persist.tile([128, BT], mmdt)
            attnT_s = persist.tile([128, HPC * BT], mmdt)
            recip_s = persist.tile([64, BT], f32)
            rT_s = persist.tile([128, NTT * HPC], f32)
            ones_s = consts.tile([128, 1], mmdt)
            identm_s = consts.tile([128, 128], mmdt)
            identf_s = consts.tile([128, 128], f32)
            nc.sync.dma_start(ones_s[:], ones[:])
            nc.sync.dma_start(identm_s[:], identm[:])
            nc.sync.dma_start(identf_s[:], identf[:])

            with tc.tile_pool(name="vt", bufs=1) as vtp:
                vT_s = vtp.tile([128, BT], mmdt)
                with (
                    tc.tile_pool(name="wgt", bufs=1) as wgt,
                    tc.tile_pool(name="xin", bufs=3) as xin,
                    tc.tile_pool(name="qkvp", bufs=2, space="PSUM") as qkvp,
                ):
                    wq_s = wgt.tile([128, NK * HPC * DH], mmdt)
                    wk_s = wgt.tile([128, NK * DH], mmdt)
                    wv_s = wgt.tile([128, NK * DH], mmdt)
                    nc.sync.dma_start(
                        wq_s[:].rearrange("p (t m) -> p t m", t=NK),
                        wq[:].rearrange("(t p) m -> p t m", p=128),
                    )
                    nc.sync.dma_start(
                        wk_s[:].rearrange("p (t m) -> p t m", t=NK),
                        wk[:].rearrange("(t p) m -> p t m", p=128),
                    )
                    nc.sync.dma_start(
                        wv_s[:].rearrange("p (t m) -> p t m", t=NK),
                        wv[:].rearrange("(t p) m -> p t m", p=128),
                    )
                    for n in range(NBT):
                        ps_q0 = qkvp.tile([128, 512], f32, tag="psq0")
                        ps_q1 = qkvp.tile([128, 512], f32, tag="psq1")
                        ps_k = qkvp.tile([128, 512], f32, tag="psk")
                        ps_v = qkvp.tile([128, 512], f32, tag="psv")
                        for kd in range(NK):
                            xt = xin.tile([128, 512], mmdt, tag="xt")
                            nc.sync.dma_start(
                                xt[:],
                                xT[kd * 128 : (kd + 1) * 128, n * 512 : (n + 1) * 512],
                            )
                            st, sp = kd == 0, kd == NK - 1
                            nc.tensor.matmul(
                                ps_q0[:], wq_s[:, kd * 256 : kd * 256 + 128], xt[:],
                                start=st, stop=sp,
                            )
                            nc.tensor.matmul(
                                ps_q1[:], wq_s[:, kd * 256 + 128 : kd * 256 + 256], xt[:],
                                start=st, stop=sp,
                            )
                            nc.tensor.matmul(
                                ps_k[:], wk_s[:, kd * 128 : (kd + 1) * 128], xt[:],
                                start=st, stop=sp,
                            )
                            nc.tensor.matmul(
                                ps_v[:], wv_s[:, kd * 128 : (kd + 1) * 128], xt[:],
                                start=st, stop=sp,
                            )
                        sl = slice(n * 512, (n + 1) * 512)
                        nc.vector.tensor_copy(qT_s[:, n * 512 : (n + 1) * 512], ps_q0[:])
                        nc.vector.tensor_copy(
                            qT_s[:, BT + n * 512 : BT + (n + 1) * 512], ps_q1[:]
                        )
                        nc.scalar.activation(kT_s[:, sl], ps_k[:], Copy)
                        nc.scalar.activation(vT_s[:, sl], ps_v[:], Copy)

                with tc.tile_pool(name="trp", bufs=4, space="PSUM") as trp:
                    for t in range(NTT):
                        tr = trp.tile([128, 128], mmdt, tag="tr")
                        nc.tensor.transpose(
                            tr[:], vT_s[:, t * 128 : (t + 1) * 128], identm_s[:]
                        )
                        nc.vector.tensor_copy(v_s[:, t * 128 : (t + 1) * 128], tr[:])

            with (
                tc.tile_pool(name="epool", bufs=20) as epool,
                tc.tile_pool(name="mpool", bufs=3) as mpool,
                tc.tile_pool(name="stp", bufs=4, space="PSUM") as stp,
                tc.tile_pool(name="pvp", bufs=2, space="PSUM") as pvp,
                tc.tile_pool(name="sump", bufs=2, space="PSUM") as sump,
            ):
                chunks = [
                    (h, b, qc)
                    for h in range(HPC)
                    for b in range(B)
                    for qc in range(NQC)
                ]

                def emit_scores(ci, kk):
                    h, b, qc = chunks[ci]
                    qsl = qT_s[
                        :,
                        h * BT + b * T + qc * 512 : h * BT + b * T + (qc + 1) * 512,
                    ]
                    ps_st = stp.tile([128, 512], f32, tag="st", name=f"st_{ci}_{kk}")
                    nc.tensor.matmul(
                        ps_st[:],
                        kT_s[:, b * T + kk * 128 : b * T + (kk + 1) * 128],
                        qsl,
                        start=True, stop=True,
                    )
                    if use_mask:
                        mt = mpool.tile([128, 512], f32, tag="mt", name=f"mt_{ci}_{kk}")
                        nc.sync.dma_start(
                            mt[:],
                            maskT[
                                kk * 128 : (kk + 1) * 128,
                                qc * 512 : (qc + 1) * 512,
                            ],
                        )
                        nc.vector.tensor_add(ps_st[:], ps_st[:], mt[:])
                    e = epool.tile([128, 512], mmdt, tag="e", name=f"e_{ci}_{kk}")
                    nc.scalar.activation(e[:], ps_st[:], Exp, scale=SCALE)
                    return e

                es_cur = [emit_scores(0, kk) for kk in range(NKK)]
                for ci in range(len(chunks)):
                    h, b, qc = chunks[ci]
                    ps_pv = pvp.tile([128, 512], f32, tag="pv", name=f"pv_{ci}")
                    ps_sum = sump.tile([1, 512], f32, tag="sum", name=f"sum_{ci}")
                    es_next = []
                    for kk in range(NKK):
                        st, sp = kk == 0, kk == NKK - 1
                        nc.tensor.matmul(
                            ps_pv[:],
                            v_s[:, (b * NKK + kk) * 128 : (b * NKK + kk + 1) * 128],
                            es_cur[kk][:],
                            start=st, stop=sp,
                        )
                        nc.tensor.matmul(
                            ps_sum[:], ones_s[:], es_cur[kk][:],
                            start=st, stop=sp,
                        )
                        if ci + 1 < len(chunks):
                            es_next.append(emit_scores(ci + 1, kk))
                    osl = slice(b * T + qc * 512, b * T + (qc + 1) * 512)
                    nc.vector.reciprocal(
                        recip_s[h * 32 : h * 32 + 1, osl], ps_sum[0:1, :]
                    )
                    nc.scalar.activation(
                        attnT_s[:, h * BT + b * T + qc * 512 : h * BT + b * T + (qc + 1) * 512],
                        ps_pv[:],
                        Copy,
                    )
                    es_cur = es_next

            with tc.tile_pool(name="rtp", bufs=4, space="PSUM") as rtp:
                for i in range(NTT):
                    tr = rtp.tile([128, 64], f32, tag="rtr")
                    nc.tensor.transpose(
                        tr[:],
                        recip_s[0:64, i * 128 : (i + 1) * 128],
                        identf_s[0:64, 0:64],
                    )
                    nc.vector.tensor_copy(rT_s[:, i * HPC : i * HPC + 1], tr[:, 0:1])
                    nc.vector.tensor_copy(
                        rT_s[:, i * HPC + 1 : i * HPC + 2], tr[:, 32:33]
                    )

            with (
                tc.tile_pool(name="wop", bufs=1) as wop,
                tc.tile_pool(name="omg", bufs=4) as omg,
                tc.tile_pool(name="wops", bufs=4, space="PSUM") as wops,
            ):
                woT_s = wop.tile([128, HPC * D_MODEL], mmdt)
                for h in range(HPC):
                    nc.sync.dma_start(
                        woT_s[:, h * D_MODEL : (h + 1) * D_MODEL],
                        woT[h * 128 : (h + 1) * 128, :],
                    )
                for i in range(NTT):
                    for dc in range(4):
                        p0 = wops.tile([128, 512], f32, tag="p0")
                        p1 = wops.tile([128, 512], f32, tag="p1")
                        nc.tensor.matmul(
                            p0[:],
                            attnT_s[:, 0 * BT + i * 128 : 0 * BT + (i + 1) * 128],
                            woT_s[:, 0 * D_MODEL + dc * 512 : 0 * D_MODEL + (dc + 1) * 512],
                            start=True, stop=True,
                        )
                        nc.tensor.matmul(
                            p1[:],
                            attnT_s[:, 1 * BT + i * 128 : 1 * BT + (i + 1) * 128],
                            woT_s[:, 1 * D_MODEL + dc * 512 : 1 * D_MODEL + (dc + 1) * 512],
                            start=True, stop=True,
                        )
                        t0 = omg.tile([128, 512], f32, tag="t0")
                        t1 = omg.tile([128, 512], f32, tag="t1")
                        nc.scalar.activation(
                            t0[:], p0[:], Copy, scale=rT_s[:, i * HPC : i * HPC + 1]
                        )
                        nc.vector.tensor_scalar_mul(
                            t1[:], p1[:], rT_s[:, i * HPC + 1 : i * HPC + 2]
                        )
                        oo = omg.tile([128, 512], f32, tag="oo")
                        nc.vector.tensor_add(oo[:], t0[:], t1[:])
                        nc.sync.dma_start(
                            out[i * 128 : (i + 1) * 128, dc * 512 : (dc + 1) * 512],
                            oo[:],
                        )
    _split_multi_waits(nc)
    return nc


_cache = {}


def _get_nc_v2():
    if "v2" not in _cache:
        _cache["v2"] = _build_v2()
    return _cache["v2"]


def _get_nc_v1(mmdt_name, use_mask):
    key = ("v1", mmdt_name, use_mask)
    if key not in _cache:
        _cache[key] = _build_v1(getattr(mybir.dt, mmdt_name), use_mask)
    return _cache[key]


def _np_dt(mmdt_name):
    if mmdt_name == "bfloat16":
        import ml_dtypes

        return ml_dtypes.bfloat16
    return np.float32


def _prep_inputs_v1(x, attn_mask, Wq1, Wq2, Wk1, Wk2, Wv1, Wv2, Wo, mmdt_name):
    ndt = _np_dt(mmdt_name)
    xT = np.ascontiguousarray(x.reshape(BT, D_MODEL).T).astype(ndt)
    identm = np.eye(128, dtype=np.float32).astype(ndt)
    identf = np.eye(128, dtype=np.float32)
    ones = np.ones((128, 1), np.float32).astype(ndt)
    use_mask = bool(np.any(attn_mask))
    maskT = None
    if use_mask:
        maskT = np.ascontiguousarray(attn_mask[0, 0].T * np.sqrt(DH)).astype(
            np.float32
        )
    Wq1_64, Wq2_64 = Wq1.astype(np.float64), Wq2.astype(np.float64)
    Wk1_64, Wk2_64 = Wk1.astype(np.float64), Wk2.astype(np.float64)
    Wv1_64, Wv2_64 = Wv1.astype(np.float64), Wv2.astype(np.float64)
    in_maps = []
    for c in range(NCORES):
        h0 = c * HPC
        kv = h0 // GROUP
        wq_f = (Wq2_64[h0 * DH : (h0 + HPC) * DH] @ Wq1_64).T
        wk_f = (Wk2_64[kv * DH : (kv + 1) * DH] @ Wk1_64).T
        wv_f = (Wv2_64[kv * DH : (kv + 1) * DH] @ Wv1_64).T
        woT_c = np.ascontiguousarray(Wo[:, h0 * DH : (h0 + HPC) * DH].T)
        m = {
            "xT": xT,
            "wq": np.ascontiguousarray(wq_f).astype(ndt),
            "wk": np.ascontiguousarray(wk_f).astype(ndt),
            "wv": np.ascontiguousarray(wv_f).astype(ndt),
            "woT": woT_c.astype(ndt),
            "ones": ones,
            "identm": identm,
            "identf": identf,
        }
        if use_mask:
            m["maskT"] = maskT
        in_maps.append(m)
    return in_maps, use_mask


def run(x, attn_mask, Wq1, Wq2, Wk1, Wk2, Wv1, Wv2, Wo, **spmd_kwargs):
    use_mask = bool(np.any(attn_mask))
    force_v1 = os.environ.get("BASS_MLA_FORCE_V1") == "1"
    if not use_mask and not force_v1:
        return _run_v2(
            {"x": x, "Wq1": Wq1, "Wq2": Wq2, "Wk1": Wk1, "Wk2": Wk2,
             "Wv1": Wv1, "Wv2": Wv2, "Wo": Wo},
            **spmd_kwargs,
        )
    mmdt_name = os.environ.get("BASS_MLA_DT", "float32r")
    in_maps, use_mask = _prep_inputs_v1(
        x, attn_mask, Wq1, Wq2, Wk1, Wk2, Wv1, Wv2, Wo, mmdt_name
    )
    nc = _get_nc_v1(mmdt_name, use_mask)
    res = bass_utils.run_bass_kernel_spmd(
        nc, in_maps, core_ids=list(range(NCORES)), **spmd_kwargs
    )
    acc = res.results[0]["out"].astype(np.float64)
    for r in res.results[1:]:
        acc += r["out"]
    out = acc.astype(np.float32).reshape(B, T, D_MODEL)
    return out, res


def kernel(x, attn_mask, Wq1, Wq2, Wk1, Wk2, Wv1, Wv2, Wo):
    out, _ = run(x, attn_mask, Wq1, Wq2, Wk1, Wk2, Wv1, Wv2, Wo)
    return out

